# revision 11
# baseline (speedup 1.0000x reference)
"""BalancedBatchNorm2d Trainium2 kernel.

Math: the reference's per-class segment-sum collapses algebraically:
  mean[c]  = (1/(L*HW)) * sum_b w_b * sum_hw X[b,c,:,:],  w_b = 1/count(label_b)
  var[c]   = E[X^2] - 2*mean*E[X] + mean^2   (plain moments over (B,HW))
  Y        = X*scale[c] + bias[c],  scale = gamma/sqrt(var+eps), bias = beta - mean*scale

Sharding: channels across the 8 cores (8 ch/core) -> every core owns all
batches for its channels, so all reductions are core-local (no collectives).

Per-core layout: x[NT=32, 128, HW=1024] f32 where partition p = b_lo*8 + ch
(16 batches x 8 channels per tile). Engine plan:
  SP   : 32 tile loads -> (after per-tile normalize) 32 stores (HWDGE ring 1)
  ACT  : consts load (HWDGE ring 2), per-tile plain sum (Copy + accum_out),
         the one sqrt
  DVE  : per-tile sum-of-squares (scalar_tensor_tensor x*x + accum_out),
         finalize algebra (reads PSUM directly), per-tile fused normalize
         (tensor_scalar mult+subtract, in place)
  PE   : one [128x128]@[128,3] matmul vs the channel-group selector matrix:
         cross-partition per-channel sums, broadcast to all partitions.
"""

import numpy as np

import concourse.bass as bass
from concourse import mybir
from concourse.bass_utils import run_bass_kernel_spmd

B, C, H, W = 512, 64, 32, 32
HW = H * W
L = 100  # num classes
EPS = 1e-6
NCORES = 8
CPC = C // NCORES  # channels per core = 8
BPT = 128 // CPC  # batches per tile = 16
NT = B // BPT  # tiles per core = 32
F32 = mybir.dt.float32

# consts tensor column layout
NCOL = NT + 128 + 3
COL_RSEL = NT
COL_G = NT + 128
COL_B = NT + 129
COL_EPS = NT + 130

_NC_CACHE = {}


def _bcast0(col_ap, n):
    # [128,1] column AP -> [128,n] write AP with free-dim stride 0 (sink).
    return bass.AP(
        tensor=col_ap.tensor,
        offset=col_ap.offset,
        ap=[list(col_ap.ap[0]), [0, n]],
    )


# load-sem grouping: coarse 4-tile groups early (fewer sems), fine at the tail
# (keeps end-of-load-phase consumption lag at one tile).
_GROUPS = [list(range(g, g + 4)) for g in range(0, 28, 4)] + [[28, 29], [30], [31]]
_TILE_SEM = {}
for _gi, _g in enumerate(_GROUPS):
    for _t in _g:
        _TILE_SEM[_t] = (_gi, 16 * len(_g))
NGROUPS = len(_GROUPS)


def build_nc():
    nc = bass.Bass()
    x_d = nc.declare_dram_parameter("x", [NT, 128, HW], F32, isOutput=False)
    c_d = nc.declare_dram_parameter("consts", [128, NCOL], F32, isOutput=False)
    y_d = nc.declare_dram_parameter("y", [NT, 128, HW], F32, isOutput=True)

    from contextlib import ExitStack

    c1 = 1.0 / (L * HW)  # balanced-mean scale
    c2 = 1.0 / (B * HW)  # plain-moment scale

    with ExitStack() as ctx:
        small_sem = ctx.enter_context(nc.semaphore("small_sem"))
        load_x = [ctx.enter_context(nc.semaphore(f"load_x{g}")) for g in range(NGROUPS)]
        store_sem = ctx.enter_context(nc.semaphore("store_sem"))
        s_act = ctx.enter_context(nc.semaphore("s_act"))
        s_sq = ctx.enter_context(nc.semaphore("s_sq"))
        s_t3 = ctx.enter_context(nc.semaphore("s_t3"))
        s_pe = ctx.enter_context(nc.semaphore("s_pe"))
        s_var = ctx.enter_context(nc.semaphore("s_var"))
        s_sd = ctx.enter_context(nc.semaphore("s_sd"))
        s_norm = ctx.enter_context(nc.semaphore("s_norm"))
        dvq = ctx.enter_context(nc.semaphore("dvq"))
        x_sb = ctx.enter_context(nc.sbuf_tensor("x_sb", [128, NT, HW], F32))
        c_sb = ctx.enter_context(nc.sbuf_tensor("c_sb", [128, NCOL], F32))
        rs_col = ctx.enter_context(nc.sbuf_tensor("rs_col", [128, NT], F32))
        sq_col = ctx.enter_context(nc.sbuf_tensor("sq_col", [128, NT], F32))
        t3 = ctx.enter_context(nc.sbuf_tensor("t3", [128, 4], F32))
        junk_act = ctx.enter_context(nc.sbuf_tensor("junk_act", [128, NT], F32))
        junk_dve = ctx.enter_context(nc.sbuf_tensor("junk_dve", [128, NT], F32))
        mean_t = ctx.enter_context(nc.sbuf_tensor("mean_t", [128, 1], F32))
        ex2_t = ctx.enter_context(nc.sbuf_tensor("ex2_t", [128, 1], F32))
        a_t = ctx.enter_context(nc.sbuf_tensor("a_t", [128, 1], F32))
        nvar_t = ctx.enter_context(nc.sbuf_tensor("nvar_t", [128, 1], F32))
        sd_t = ctx.enter_context(nc.sbuf_tensor("sd_t", [128, 1], F32))
        rstd_t = ctx.enter_context(nc.sbuf_tensor("rstd_t", [128, 1], F32))
        scale_t = ctx.enter_context(nc.sbuf_tensor("scale_t", [128, 1], F32))
        nbias_t = ctx.enter_context(nc.sbuf_tensor("nbias_t", [128, 1], F32))
        p3 = ctx.enter_context(nc.psum_tensor("p3", [128, 4], F32))

        wm_ap = c_sb[:, 0:NT]
        rsel_ap = c_sb[:, COL_RSEL : COL_RSEL + 128]
        gv_ap = c_sb[:, COL_G : COL_G + 1]
        bv_ap = c_sb[:, COL_B : COL_B + 1]
        eps_ap = c_sb[:, COL_EPS : COL_EPS + 1]

        with nc.Block() as block:

            @block.sync
            def _(sp):
                for t in range(NT):
                    sp.dma_start(out=x_sb[:, t, :], in_=x_d[t]).then_inc(
                        load_x[_TILE_SEM[t][0]], 16
                    )
                for t in range(NT):
                    sp.wait_ge(s_norm, t + 1)
                    sp.dma_start(out=y_d[t], in_=x_sb[:, t, :]).then_inc(store_sem, 16)
                sp.wait_ge(store_sem, 16 * NT)

            @block.scalar
            def _(act):
                act.dma_start(out=c_sb[:, :], in_=c_d[:, :]).then_inc(small_sem, 16)
                for gi, g in enumerate(_GROUPS):
                    act.wait_ge(load_x[gi], 16 * len(g))
                    for t in g:
                        act.activation(
                            out=_bcast0(junk_act[:, t : t + 1], HW),
                            in_=x_sb[:, t, :],
                            func=mybir.ActivationFunctionType.Copy,
                            accum_out=rs_col[:, t : t + 1],
                        ).then_inc(s_act, 1)
                # rstd precursor: sd = sqrt(var + eps) = sqrt(-nvar + eps)
                act.wait_ge(small_sem, 16)
                act.wait_ge(s_var, 1)
                act.activation(
                    out=sd_t[:, :],
                    in_=nvar_t[:, :],
                    func=mybir.ActivationFunctionType.Sqrt,
                    scale=-1.0,
                    bias=eps_ap,
                ).then_inc(s_sd, 1)

            @block.vector
            def _(dve):
                for gi, g in enumerate(_GROUPS):
                    dve.wait_ge(load_x[gi], 16 * len(g))
                    for t in g:
                        dve.scalar_tensor_tensor(
                            out=_bcast0(junk_dve[:, t : t + 1], HW),
                            in0=x_sb[:, t, :],
                            scalar=1.0,
                            in1=x_sb[:, t, :],
                            op0=mybir.AluOpType.mult,
                            op1=mybir.AluOpType.mult,
                            accum_out=sq_col[:, t : t + 1],
                        ).then_inc(s_sq, 1)
                dve.wait_ge(s_act, NT)
                dve.wait_ge(s_sq, NT)
                dve.wait_ge(small_sem, 16)
                # t3 col0 = sum(w * rowsum), col1 = sum(rowsum), col2 = sum(rowsumsq)
                dve.scalar_tensor_tensor(
                    out=_bcast0(junk_dve[:, 0:1], NT),
                    in0=rs_col[:, :],
                    scalar=1.0,
                    in1=wm_ap,
                    op0=mybir.AluOpType.mult,
                    op1=mybir.AluOpType.mult,
                    accum_out=t3[:, 0:1],
                ).then_inc(s_t3, 1)
                dve.tensor_reduce(
                    out=t3[:, 1:2],
                    in_=rs_col[:, :],
                    axis=mybir.AxisListType.X,
                    op=mybir.AluOpType.add,
                ).then_inc(s_t3, 1)
                dve.tensor_reduce(
                    out=t3[:, 2:3],
                    in_=sq_col[:, :],
                    axis=mybir.AxisListType.X,
                    op=mybir.AluOpType.add,
                ).then_inc(s_t3, 1)
                # finalize algebra straight off PSUM:
                #   mean = P0*c1; ex2c = P2*c2; a = P1*2c2 - mean
                #   nvar = a*mean - ex2c = -(var)
                dve.wait_ge(s_pe, 1)
                dve.tensor_scalar_mul(mean_t[:, :], p3[:, 0:1], c1).then_inc(dvq, 1)
                dve.tensor_scalar_mul(ex2_t[:, :], p3[:, 2:3], c2).then_inc(dvq, 1)
                dve.wait_ge(dvq, 2)
                dve.scalar_tensor_tensor(
                    out=a_t[:, :],
                    in0=p3[:, 1:2],
                    scalar=2.0 * c2,
                    in1=mean_t[:, :],
                    op0=mybir.AluOpType.mult,
                    op1=mybir.AluOpType.subtract,
                ).then_inc(dvq, 1)
                dve.wait_ge(dvq, 3)
                dve.scalar_tensor_tensor(
                    out=nvar_t[:, :],
                    in0=a_t[:, :],
                    scalar=mean_t[:, :],
                    in1=ex2_t[:, :],
                    op0=mybir.AluOpType.mult,
                    op1=mybir.AluOpType.subtract,
                ).then_inc(s_var, 1)
                # rstd = 1/sd; scale = gamma*rstd; nbias = mean*scale - beta
                dve.wait_ge(s_sd, 1)
                dve.reciprocal(rstd_t[:, :], sd_t[:, :]).then_inc(dvq, 1)
                dve.wait_ge(dvq, 4)
                dve.tensor_mul(scale_t[:, :], gv_ap, rstd_t[:, :]).then_inc(dvq, 1)
                dve.wait_ge(dvq, 5)
                dve.scalar_tensor_tensor(
                    out=nbias_t[:, :],
                    in0=scale_t[:, :],
                    scalar=mean_t[:, :],
                    in1=bv_ap,
                    op0=mybir.AluOpType.mult,
                    op1=mybir.AluOpType.subtract,
                ).then_inc(dvq, 1)
                dve.wait_ge(dvq, 6)
                # y = x*scale - nbias  (in place)
                for t in range(NT):
                    dve.tensor_scalar(
                        out=x_sb[:, t, :],
                        in0=x_sb[:, t, :],
                        scalar1=scale_t[:, :],
                        scalar2=nbias_t[:, :],
                        op0=mybir.AluOpType.mult,
                        op1=mybir.AluOpType.subtract,
                    ).then_inc(s_norm, 1)

            @block.tensor
            def _(pe):
                pe.wait_ge(small_sem, 16)
                pe.wait_ge(s_t3, 3)
                pe.matmul(
                    p3[:, 0:3],
                    rsel_ap,
                    t3[:, 0:3],
                    start=True,
                    stop=True,
                ).then_inc(s_pe, 1)

    return nc


def get_nc():
    if "nc" not in _NC_CACHE:
        _NC_CACHE["nc"] = build_nc()
    return _NC_CACHE["nc"]


def make_in_maps(X, label, gamma, beta):
    """Host-side sharding: full inputs -> per-core input maps."""
    X = np.asarray(X, dtype=np.float32)
    label = np.asarray(label).astype(np.int64).ravel()
    gamma = np.asarray(gamma, dtype=np.float32).reshape(C)
    beta = np.asarray(beta, dtype=np.float32).reshape(C)

    cnt = np.bincount(label, minlength=L).astype(np.float32)
    cnt = np.maximum(cnt, 1.0)  # absent classes never indexed; avoid div0
    w = (1.0 / cnt[label]).astype(np.float32)  # (B,)

    # wmat[p, t] = w[t*BPT + p // CPC]
    wmat = np.broadcast_to(w.reshape(NT, BPT, 1), (NT, BPT, CPC)).reshape(NT, 128).T
    pch = np.arange(128) % CPC
    rsel = (pch[:, None] == pch[None, :]).astype(np.float32)

    Xr = X.reshape(B, C, HW)
    in_maps = []
    for i in range(NCORES):
        sl = slice(i * CPC, (i + 1) * CPC)
        xs = np.ascontiguousarray(Xr[:, sl, :]).reshape(NT, 128, HW)
        consts = np.empty((128, NCOL), np.float32)
        consts[:, 0:NT] = wmat
        consts[:, COL_RSEL : COL_RSEL + 128] = rsel
        consts[:, COL_G] = np.tile(gamma[sl], BPT)
        consts[:, COL_B] = np.tile(beta[sl], BPT)
        consts[:, COL_EPS] = EPS
        in_maps.append({"x": xs, "consts": consts})
    return in_maps


def assemble_output(results):
    Y = np.empty((B, C, HW), np.float32)
    for i in range(NCORES):
        Y[:, i * CPC : (i + 1) * CPC, :] = results[i]["y"].reshape(B, CPC, HW)
    return Y.reshape(B, C, H, W)


def kernel(X, label, gamma, beta):
    in_maps = make_in_maps(X, label, gamma, beta)
    nc = get_nc()
    res = run_bass_kernel_spmd(nc, in_maps, list(range(NCORES)))
    return assemble_output(res.results)


# revision 14
# speedup vs baseline: 1.0346x; 1.0346x over previous
"""BalancedBatchNorm2d Trainium2 kernel.

Math: the reference's per-class segment-sum collapses algebraically:
  mean[c]  = (1/(L*HW)) * sum_b w_b * sum_hw X[b,c,:,:],  w_b = 1/count(label_b)
  var[c]   = E[X^2] - 2*mean*E[X] + mean^2   (plain moments over (B,HW))
  Y        = X*scale[c] + bias[c],  scale = gamma/sqrt(var+eps), bias = beta - mean*scale

Sharding: channels across the 8 cores (8 ch/core) -> every core owns all
batches for its channels, so all reductions are core-local (no collectives).

Per-core layout: x[NT=32, 128, HW=1024] f32 where partition p = b_lo*8 + ch
(16 batches x 8 channels per tile). Engine plan:
  SP   : 32 tile loads -> (after per-tile normalize) 32 stores (HWDGE ring 1)
  ACT  : consts load (HWDGE ring 2), per-tile plain sum (Copy + accum_out),
         the one sqrt
  DVE  : per-tile sum-of-squares (scalar_tensor_tensor x*x + accum_out),
         finalize algebra (reads PSUM directly), per-tile fused normalize
         (tensor_scalar mult+subtract, in place)
  PE   : one [128x128]@[128,3] matmul vs the channel-group selector matrix:
         cross-partition per-channel sums, broadcast to all partitions.
"""

import numpy as np

import concourse.bass as bass
from concourse import mybir
from concourse.bass_utils import run_bass_kernel_spmd

B, C, H, W = 512, 64, 32, 32
HW = H * W
L = 100  # num classes
EPS = 1e-6
NCORES = 8
CPC = C // NCORES  # channels per core = 8
BPT = 128 // CPC  # batches per tile = 16
NT = B // BPT  # tiles per core = 32
F32 = mybir.dt.float32

# consts tensor column layout
NCOL = NT + 128 + 3
COL_RSEL = NT
COL_G = NT + 128
COL_B = NT + 129
COL_EPS = NT + 130

_NC_CACHE = {}


def _bcast0(col_ap, n):
    # [128,1] column AP -> [128,n] write AP with free-dim stride 0 (sink).
    return bass.AP(
        tensor=col_ap.tensor,
        offset=col_ap.offset,
        ap=[list(col_ap.ap[0]), [0, n]],
    )


# one completion semaphore per tile load: sem==16 is the only sound per-DMA
# completion signal (multi-DMA sems interleave their 16 per-engine increments),
# and per-tile granularity keeps the compute engines tightly chasing the loads.
_GROUPS = [[t] for t in range(NT)]
_TILE_SEM = {}
for _gi, _g in enumerate(_GROUPS):
    for _t in _g:
        _TILE_SEM[_t] = (_gi, 16 * len(_g))
NGROUPS = len(_GROUPS)


def build_nc():
    nc = bass.Bass()
    x_d = nc.declare_dram_parameter("x", [NT, 128, HW], F32, isOutput=False)
    c_d = nc.declare_dram_parameter("consts", [128, NCOL], F32, isOutput=False)
    y_d = nc.declare_dram_parameter("y", [NT, 128, HW], F32, isOutput=True)

    from contextlib import ExitStack

    c1 = 1.0 / (L * HW)  # balanced-mean scale
    c2 = 1.0 / (B * HW)  # plain-moment scale

    with ExitStack() as ctx:
        small_sem = ctx.enter_context(nc.semaphore("small_sem"))
        load_x = [ctx.enter_context(nc.semaphore(f"load_x{g}")) for g in range(NGROUPS)]
        store_sem = ctx.enter_context(nc.semaphore("store_sem"))
        s_act = ctx.enter_context(nc.semaphore("s_act"))
        s_sq = ctx.enter_context(nc.semaphore("s_sq"))
        s_t3a = ctx.enter_context(nc.semaphore("s_t3a"))
        s_t3b = ctx.enter_context(nc.semaphore("s_t3b"))
        s_pe = ctx.enter_context(nc.semaphore("s_pe"))
        s_var = ctx.enter_context(nc.semaphore("s_var"))
        s_sd = ctx.enter_context(nc.semaphore("s_sd"))
        s_norm = ctx.enter_context(nc.semaphore("s_norm"))
        dvq = ctx.enter_context(nc.semaphore("dvq"))
        x_sb = ctx.enter_context(nc.sbuf_tensor("x_sb", [128, NT, HW], F32))
        c_sb = ctx.enter_context(nc.sbuf_tensor("c_sb", [128, NCOL], F32))
        rs_col = ctx.enter_context(nc.sbuf_tensor("rs_col", [128, NT], F32))
        sq_col = ctx.enter_context(nc.sbuf_tensor("sq_col", [128, NT], F32))
        t3 = ctx.enter_context(nc.sbuf_tensor("t3", [128, 4], F32))
        t4 = ctx.enter_context(nc.sbuf_tensor("t4", [128, 4], F32))
        junk_act = ctx.enter_context(nc.sbuf_tensor("junk_act", [128, NT], F32))
        junk_dve = ctx.enter_context(nc.sbuf_tensor("junk_dve", [128, NT], F32))
        mean_t = ctx.enter_context(nc.sbuf_tensor("mean_t", [128, 1], F32))
        ex2_t = ctx.enter_context(nc.sbuf_tensor("ex2_t", [128, 1], F32))
        a_t = ctx.enter_context(nc.sbuf_tensor("a_t", [128, 1], F32))
        nvar_t = ctx.enter_context(nc.sbuf_tensor("nvar_t", [128, 1], F32))
        sd_t = ctx.enter_context(nc.sbuf_tensor("sd_t", [128, 1], F32))
        rstd_t = ctx.enter_context(nc.sbuf_tensor("rstd_t", [128, 1], F32))
        scale_t = ctx.enter_context(nc.sbuf_tensor("scale_t", [128, 1], F32))
        nbias_t = ctx.enter_context(nc.sbuf_tensor("nbias_t", [128, 1], F32))
        p3 = ctx.enter_context(nc.psum_tensor("p3", [128, 4], F32))

        wm_ap = c_sb[:, 0:NT]
        rsel_ap = c_sb[:, COL_RSEL : COL_RSEL + 128]
        gv_ap = c_sb[:, COL_G : COL_G + 1]
        bv_ap = c_sb[:, COL_B : COL_B + 1]
        eps_ap = c_sb[:, COL_EPS : COL_EPS + 1]

        with nc.Block() as block:

            @block.sync
            def _(sp):
                for t in range(NT):
                    sp.dma_start(out=x_sb[:, t, :], in_=x_d[t]).then_inc(
                        load_x[_TILE_SEM[t][0]], 16
                    )
                for t in range(NT):
                    sp.wait_ge(s_norm, t + 1)
                    sp.dma_start(out=y_d[t], in_=x_sb[:, t, :]).then_inc(store_sem, 16)
                sp.wait_ge(store_sem, 16 * NT)

            @block.scalar
            def _(act):
                act.dma_start(out=c_sb[:, :], in_=c_d[:, :]).then_inc(small_sem, 16)
                for gi, g in enumerate(_GROUPS):
                    act.wait_ge(load_x[gi], 16 * len(g))
                    for t in g:
                        act.activation(
                            out=_bcast0(junk_act[:, t : t + 1], HW),
                            in_=x_sb[:, t, :],
                            func=mybir.ActivationFunctionType.Copy,
                            accum_out=rs_col[:, t : t + 1],
                        ).then_inc(s_act, 1)
                # rstd precursor: sd = sqrt(var + eps) = sqrt(-nvar + eps)
                act.wait_ge(small_sem, 16)
                act.wait_ge(s_var, 1)
                act.activation(
                    out=sd_t[:, :],
                    in_=nvar_t[:, :],
                    func=mybir.ActivationFunctionType.Sqrt,
                    scale=-1.0,
                    bias=eps_ap,
                ).then_inc(s_sd, 1)

            @block.vector
            def _(dve):
                def _sumsq(t):
                    dve.scalar_tensor_tensor(
                        out=_bcast0(junk_dve[:, t : t + 1], HW),
                        in0=x_sb[:, t, :],
                        scalar=1.0,
                        in1=x_sb[:, t, :],
                        op0=mybir.AluOpType.mult,
                        op1=mybir.AluOpType.mult,
                        accum_out=sq_col[:, t : t + 1],
                    ).then_inc(s_sq, 1)

                def _t3cols(dst, sem, sl, n):
                    # dst col0=sum(w*rowsum), col1=sum(rowsum), col2=sum(rowsumsq)
                    dve.scalar_tensor_tensor(
                        out=_bcast0(junk_dve[:, 0:1], n),
                        in0=rs_col[:, sl],
                        scalar=1.0,
                        in1=wm_ap[:, sl],
                        op0=mybir.AluOpType.mult,
                        op1=mybir.AluOpType.mult,
                        accum_out=dst[:, 0:1],
                    ).then_inc(sem, 1)
                    dve.tensor_reduce(
                        out=dst[:, 1:2],
                        in_=rs_col[:, sl],
                        axis=mybir.AxisListType.X,
                        op=mybir.AluOpType.add,
                    ).then_inc(sem, 1)
                    dve.tensor_reduce(
                        out=dst[:, 2:3],
                        in_=sq_col[:, sl],
                        axis=mybir.AxisListType.X,
                        op=mybir.AluOpType.add,
                    ).then_inc(sem, 1)

                SPLIT = 28
                for t in range(SPLIT):
                    dve.wait_ge(load_x[_TILE_SEM[t][0]], _TILE_SEM[t][1])
                    _sumsq(t)
                # partial stats over tiles [0, SPLIT) hide under the load tail
                dve.wait_ge(s_act, SPLIT)
                dve.wait_ge(s_sq, SPLIT)
                dve.wait_ge(small_sem, 16)
                _t3cols(t3, s_t3a, slice(0, SPLIT), SPLIT)
                for t in range(SPLIT, NT):
                    dve.wait_ge(load_x[_TILE_SEM[t][0]], _TILE_SEM[t][1])
                    _sumsq(t)
                dve.wait_ge(s_act, NT)
                dve.wait_ge(s_sq, NT)
                _t3cols(t4, s_t3b, slice(SPLIT, NT), NT - SPLIT)
                # finalize algebra straight off PSUM:
                #   mean = P0*c1; ex2c = P2*c2; a = P1*2c2 - mean
                #   nvar = a*mean - ex2c = -(var)
                dve.wait_ge(s_pe, 2)
                dve.tensor_scalar_mul(mean_t[:, :], p3[:, 0:1], c1).then_inc(dvq, 1)
                dve.tensor_scalar_mul(ex2_t[:, :], p3[:, 2:3], c2).then_inc(dvq, 1)
                dve.wait_ge(dvq, 2)
                dve.scalar_tensor_tensor(
                    out=a_t[:, :],
                    in0=p3[:, 1:2],
                    scalar=2.0 * c2,
                    in1=mean_t[:, :],
                    op0=mybir.AluOpType.mult,
                    op1=mybir.AluOpType.subtract,
                ).then_inc(dvq, 1)
                dve.wait_ge(dvq, 3)
                dve.scalar_tensor_tensor(
                    out=nvar_t[:, :],
                    in0=a_t[:, :],
                    scalar=mean_t[:, :],
                    in1=ex2_t[:, :],
                    op0=mybir.AluOpType.mult,
                    op1=mybir.AluOpType.subtract,
                ).then_inc(s_var, 1)
                # rstd = 1/sd; scale = gamma*rstd; nbias = mean*scale - beta
                dve.wait_ge(s_sd, 1)
                dve.reciprocal(rstd_t[:, :], sd_t[:, :]).then_inc(dvq, 1)
                dve.wait_ge(dvq, 4)
                dve.tensor_mul(scale_t[:, :], gv_ap, rstd_t[:, :]).then_inc(dvq, 1)
                dve.wait_ge(dvq, 5)
                dve.scalar_tensor_tensor(
                    out=nbias_t[:, :],
                    in0=scale_t[:, :],
                    scalar=mean_t[:, :],
                    in1=bv_ap,
                    op0=mybir.AluOpType.mult,
                    op1=mybir.AluOpType.subtract,
                ).then_inc(dvq, 1)
                dve.wait_ge(dvq, 6)
                # y = x*scale - nbias  (in place)
                for t in range(NT):
                    dve.tensor_scalar(
                        out=x_sb[:, t, :],
                        in0=x_sb[:, t, :],
                        scalar1=scale_t[:, :],
                        scalar2=nbias_t[:, :],
                        op0=mybir.AluOpType.mult,
                        op1=mybir.AluOpType.subtract,
                    ).then_inc(s_norm, 1)

            @block.tensor
            def _(pe):
                pe.wait_ge(small_sem, 16)
                pe.wait_ge(s_t3a, 3)
                pe.matmul(
                    p3[:, 0:3],
                    rsel_ap,
                    t3[:, 0:3],
                    start=True,
                    stop=False,
                ).then_inc(s_pe, 1)
                pe.wait_ge(s_t3b, 3)
                pe.matmul(
                    p3[:, 0:3],
                    rsel_ap,
                    t4[:, 0:3],
                    start=False,
                    stop=True,
                ).then_inc(s_pe, 1)

    return nc


def get_nc():
    if "nc" not in _NC_CACHE:
        _NC_CACHE["nc"] = build_nc()
    return _NC_CACHE["nc"]


def make_in_maps(X, label, gamma, beta):
    """Host-side sharding: full inputs -> per-core input maps."""
    X = np.asarray(X, dtype=np.float32)
    label = np.asarray(label).astype(np.int64).ravel()
    gamma = np.asarray(gamma, dtype=np.float32).reshape(C)
    beta = np.asarray(beta, dtype=np.float32).reshape(C)

    cnt = np.bincount(label, minlength=L).astype(np.float32)
    cnt = np.maximum(cnt, 1.0)  # absent classes never indexed; avoid div0
    w = (1.0 / cnt[label]).astype(np.float32)  # (B,)

    # wmat[p, t] = w[t*BPT + p // CPC]
    wmat = np.broadcast_to(w.reshape(NT, BPT, 1), (NT, BPT, CPC)).reshape(NT, 128).T
    pch = np.arange(128) % CPC
    rsel = (pch[:, None] == pch[None, :]).astype(np.float32)

    Xr = X.reshape(B, C, HW)
    in_maps = []
    for i in range(NCORES):
        sl = slice(i * CPC, (i + 1) * CPC)
        xs = np.ascontiguousarray(Xr[:, sl, :]).reshape(NT, 128, HW)
        consts = np.empty((128, NCOL), np.float32)
        consts[:, 0:NT] = wmat
        consts[:, COL_RSEL : COL_RSEL + 128] = rsel
        consts[:, COL_G] = np.tile(gamma[sl], BPT)
        consts[:, COL_B] = np.tile(beta[sl], BPT)
        consts[:, COL_EPS] = EPS
        in_maps.append({"x": xs, "consts": consts})
    return in_maps


def assemble_output(results):
    Y = np.empty((B, C, HW), np.float32)
    for i in range(NCORES):
        Y[:, i * CPC : (i + 1) * CPC, :] = results[i]["y"].reshape(B, CPC, HW)
    return Y.reshape(B, C, H, W)


def kernel(X, label, gamma, beta):
    in_maps = make_in_maps(X, label, gamma, beta)
    nc = get_nc()
    res = run_bass_kernel_spmd(nc, in_maps, list(range(NCORES)))
    return assemble_output(res.results)
